# revision 1
# baseline (speedup 1.0000x reference)
"""CrossAssetGNN forward on 8 Trainium2 cores, data-parallel over batch.

Key algebraic reductions vs the reference:
- Only the last 15 timesteps of x feed the output (causal dilated convs,
  receptive field 15, last-timestep readout) -> upload/compute 15/128 of x.
- The gather/scatter GAT over E=16770 random edges collapses to dense
  130x130 ops via a host-precomputed edge-count matrix C[dst,src]:
  every per-edge quantity depends only on (src,dst), so duplicate edges
  fold into integer multiplicities. The softmax max-subtraction cancels
  (up to the 1e-8 epsilon, ~1e-10 relative) and is dropped.
- Edge-weight MLP is evaluated densely for all (dst,src) pairs with the
  relu'd pre-activation block as the *stationary* matmul operand so each
  result column lands partition-parallel in PSUM.
"""
import json
import sys

sys.path.insert(0, "/opt/trn_rl_repo")

import numpy as np
from contextlib import ExitStack

import concourse.bass as bass
import concourse.tile as tile
from concourse import masks, mybir
from concourse.bass_utils import run_bass_kernel_spmd

f32 = mybir.dt.float32
AF = mybir.ActivationFunctionType
OP = mybir.AluOpType

B, A, AUX, T, DIN, H, ODIM = 32, 128, 2, 128, 64, 128, 3
N = A + AUX            # 130
NC_CORES = 8
NB = B // NC_CORES     # 4 graphs per core
W = 15                 # receptive field of the three causal convs
BN_EPS = 1e-5
NCOL = NB * N          # 520 node columns per core
JBLK = 26              # j's per dense edge-MLP block


# ---- walrus workaround: max 1 sync-wait command per instruction ------------
def _apply_sync_split_patch():
    if getattr(bass.Bass, "_sync_split_patched", False):
        return
    orig = bass.Bass.to_json_bytes

    def to_json_bytes(self, *a, **kw):
        m = json.loads(orig(self, *a, **kw))
        for f in m.get("functions", []):
            for blk in f.get("blocks", []):
                new = []
                for inst in blk.get("instructions", []):
                    si = inst.get("sync_info")
                    if (si and si.get("on_wait") and len(si["on_wait"]) > 1
                            and inst.get("engine") in
                            {"PE", "DVE", "Activation", "SP", "Pool"}):
                        waits = si["on_wait"]
                        for k, w in enumerate(waits[:-1]):
                            new.append({"engine": inst["engine"], "ins": [],
                                        "outs": [],
                                        "name": f"{inst['name']}-sw{k}",
                                        "opcode": "NoOp",
                                        "sync_info": {"on_update": [],
                                                      "on_wait": [w]}})
                        si["on_wait"] = waits[-1:]
                    new.append(inst)
                blk["instructions"] = new
        return json.dumps(m).encode()

    bass.Bass.to_json_bytes = to_json_bytes
    bass.Bass._sync_split_patched = True


def _bcast_ap(t, offset_elems, dims):
    """AP over SBUF tile t: partition dim + given free [step, count] dims."""
    return bass.AP(tensor=t.tensor, offset=t.offset + offset_elems,
                   ap=[list(t.ap[0])] + [list(d) for d in dims])


def _chunks(total, step):
    return [(s, min(step, total - s)) for s in range(0, total, step)]


def build_program():
    nc = bass.Bass("TRN2", target_bir_lowering=False, num_devices=NC_CORES)

    din = {}

    def d_in(name, shape):
        din[name] = nc.dram_tensor(name, list(shape), f32, kind="ExternalInput")
        return din[name]

    d_in("xt", [DIN, NCOL * W])
    d_in("W_embT", [DIN, H]); d_in("b_emb", [H, 1])
    d_in("cw_all", [H, 9 * H]); d_in("sc_all", [H, 3]); d_in("bi_all", [H, 3])
    d_in("W1aT", [H, H]); d_in("W1bT", [H, H]); d_in("b1", [H, 1])
    d_in("w2", [H, 1])
    d_in("CA", [128, N]); d_in("CB", [2, N])
    d_in("gWT", [H, 3 * H]); d_in("asrc", [H, 3]); d_in("adst", [H, 3])
    d_in("hW1T", [H, A * 64]); d_in("b1exp", [64, A * NB])
    d_in("hW2T", [64, A * ODIM]); d_in("b2exp", [ODIM, A * NB])
    d_in("b2ew", [1, 1])

    o_logits = nc.dram_tensor("logits", [ODIM, A * NB], f32, kind="ExternalOutput")
    o_probs = nc.dram_tensor("probs", [128, NB * ODIM], f32, kind="ExternalOutput")

    with tile.TileContext(nc) as tc:
        with ExitStack() as top:
            const = top.enter_context(tc.tile_pool(name="const", bufs=1))
            persist = top.enter_context(tc.tile_pool(name="persist", bufs=1))

            def load(name, shape):
                t = const.tile(list(shape), f32, name=f"c_{name}", tag=f"c_{name}")
                nc.sync.dma_start(out=t, in_=din[name][:, :])
                return t

            W_embT = load("W_embT", [DIN, H]); b_emb = load("b_emb", [H, 1])
            cw_all = load("cw_all", [H, 9 * H])
            sc_all = load("sc_all", [H, 3]); bi_all = load("bi_all", [H, 3])
            W1aT = load("W1aT", [H, H]); W1bT = load("W1bT", [H, H])
            b1 = load("b1", [H, 1]); w2 = load("w2", [H, 1])
            CAt = load("CA", [128, N]); CBt = load("CB", [2, N])
            gWT = load("gWT", [H, 3 * H])
            asrc = load("asrc", [H, 3]); adst = load("adst", [H, 3])
            hW1T = load("hW1T", [H, A * 64]); b1exp = load("b1exp", [64, A * NB])
            hW2T = load("hW2T", [64, A * ODIM]); b2exp = load("b2exp", [ODIM, A * NB])
            b2ap = din["b2ew"][:, :]
            b2col = const.tile([128, 1], f32)
            nc.sync.dma_start(out=b2col, in_=bass.AP(
                tensor=b2ap.tensor, offset=b2ap.offset, ap=[[0, 128], [1, 1]]))

            ident = const.tile([128, 128], f32)
            masks.make_identity(nc, ident[:, :])
            alpha02 = const.tile([128, 1], f32)
            nc.vector.memset(alpha02[:, :], 0.2)
            ones_row = const.tile([1, NCOL], f32)
            nc.vector.memset(ones_row[:, :], 1.0)

            feats = persist.tile([H, NCOL], f32)

            # ---------------- stage A: embed + 3 dilated causal convs -------
            with ExitStack() as sA:
                front = sA.enter_context(tc.tile_pool(name="front", bufs=1))
                psA = sA.enter_context(
                    tc.tile_pool(name="psA", bufs=3, space="PSUM"))

                xT = front.tile([DIN, NCOL * W], f32)
                nc.sync.dma_start(out=xT, in_=din["xt"][:, :])
                emb = front.tile([H, NCOL * W], f32)
                for s, ln in _chunks(NCOL * W, 512):
                    pe = psA.tile([128, 512], f32, tag="pe")
                    nc.tensor.matmul(pe[:, :ln], lhsT=W_embT[:, :],
                                     rhs=xT[:, s:s + ln], start=True, stop=True)
                    nc.scalar.activation(emb[:, s:s + ln], pe[:, :ln],
                                         AF.Identity, bias=b_emb[:, :])

                # conv layers: (out_len per block, in_len, dilation)
                l1 = front.tile([H, NCOL * 13], f32)
                l2 = front.tile([H, NCOL * 9], f32)
                convs = [(emb, W, 13, 1, 0, l1), (l1, 13, 9, 2, 1, l2),
                         (l2, 9, 1, 4, 2, feats)]
                for src, in_len, out_len, dil, li, dst in convs:
                    sv = src.rearrange("p (blk t) -> p blk t", t=in_len)
                    bpc = max(1, 507 // out_len)
                    for b0, nb in _chunks(NCOL, bpc):
                        pe = psA.tile([128, 512], f32, tag="pe")
                        w_cols = nb * out_len
                        for k in range(3):
                            rhs = sv[:, b0:b0 + nb,
                                     k * dil:k * dil + out_len]
                            nc.tensor.matmul(
                                pe[:, :w_cols],
                                lhsT=cw_all[:, (li * 3 + k) * H:(li * 3 + k + 1) * H],
                                rhs=rhs, start=(k == 0), stop=(k == 2))
                        nc.scalar.activation(
                            dst[:, b0 * out_len:b0 * out_len + w_cols],
                            pe[:, :w_cols], AF.Gelu,
                            bias=bi_all[:, li:li + 1], scale=sc_all[:, li:li + 1])

            # ---------------- stage B: dense edge-weight MLP ----------------
            ewA = [persist.tile([128, N], f32, name=f"ewA{b}", tag=f"ewA{b}")
                   for b in range(NB)]
            ewB = [persist.tile([2, N], f32, name=f"ewB{b}", tag=f"ewB{b}")
                   for b in range(NB)]
            with ExitStack() as sB:
                ewk = sB.enter_context(tc.tile_pool(name="ewk", bufs=3))
                psU = sB.enter_context(tc.tile_pool(name="psU", bufs=2, space="PSUM"))
                psE = sB.enter_context(tc.tile_pool(name="psE", bufs=2, space="PSUM"))

                Ut = persist.tile([H, NCOL], f32)
                Vt = persist.tile([H, NCOL], f32)
                for s, ln in _chunks(NCOL, 512):
                    pu = psU.tile([128, 512], f32, tag="uv")
                    nc.tensor.matmul(pu[:, :ln], lhsT=W1aT[:, :],
                                     rhs=feats[:, s:s + ln], start=True, stop=True)
                    nc.vector.tensor_copy(Ut[:, s:s + ln], pu[:, :ln])
                    pv = psU.tile([128, 512], f32, tag="uv")
                    nc.tensor.matmul(pv[:, :ln], lhsT=W1bT[:, :],
                                     rhs=feats[:, s:s + ln], start=True, stop=True)
                    nc.scalar.activation(Vt[:, s:s + ln], pv[:, :ln],
                                         AF.Identity, bias=b1[:, :])

                for b in range(NB):
                    pA = psE.tile([128, N], f32, tag="ewpsA")
                    pB = psE.tile([2, N], f32, tag="ewpsB")
                    for jb in range(N // JBLK):
                        R = ewk.tile([128, JBLK * N], f32, tag="R")
                        in0 = _bcast_ap(Ut, b * N + jb * JBLK, [[1, JBLK], [0, N]])
                        in1 = _bcast_ap(Vt, b * N, [[0, JBLK], [1, N]])
                        nc.vector.tensor_tensor(out=R[:, :], in0=in0, in1=in1,
                                                op=OP.add)
                        nc.scalar.activation(R[:, :], R[:, :], AF.Relu)
                        for jl in range(JBLK):
                            j = jb * JBLK + jl
                            nc.tensor.matmul(pA[:, j:j + 1],
                                             lhsT=R[:, jl * N:jl * N + 128],
                                             rhs=w2[:, :], start=True, stop=True)
                            nc.tensor.matmul(pB[:, j:j + 1],
                                             lhsT=R[:, jl * N + 128:jl * N + N],
                                             rhs=w2[:, :], start=True, stop=True)
                    nc.scalar.activation(ewA[b][:, :], pA[:, :], AF.Sigmoid,
                                         bias=b2col[:, :])
                    nc.scalar.activation(ewB[b][:, :], pB[:, :], AF.Sigmoid,
                                         bias=b2col[0:2, :])

            # ---------------- stage C: 3 dense GAT layers -------------------
            nfT = feats
            with ExitStack() as sC:
                gw = sC.enter_context(tc.tile_pool(name="gw", bufs=2))
                gps = sC.enter_context(tc.tile_pool(name="gps", bufs=1, space="PSUM"))
                gsq = sC.enter_context(tc.tile_pool(name="gsq", bufs=2, space="PSUM"))

                for li in range(3):
                    gW = gWT[:, li * H:(li + 1) * H]
                    hpT = gw.tile([H, NCOL], f32, tag="hpT")
                    for s, ln in _chunks(NCOL, 512):
                        ph = gps.tile([128, 512], f32, tag="big")
                        nc.tensor.matmul(ph[:, :ln], lhsT=gW, rhs=nfT[:, s:s + ln],
                                         start=True, stop=True)
                        nc.vector.tensor_copy(hpT[:, s:s + ln], ph[:, :ln])

                    as_sb = gw.tile([1, NCOL], f32, tag="as")
                    ad_sb = gw.tile([1, NCOL], f32, tag="ad")
                    for col, vec, dst in ((0, asrc, as_sb), (1, adst, ad_sb)):
                        pav = gsq.tile([1, NCOL], f32, tag="arow", bufs=1)
                        for s, ln in _chunks(NCOL, 512):
                            nc.tensor.matmul(pav[0:1, s:s + ln],
                                             lhsT=vec[:, li:li + 1],
                                             rhs=hpT[:, s:s + ln],
                                             start=True, stop=True)
                        nc.vector.tensor_copy(dst[:, :], pav[:, :])

                    R2 = gw.tile([2, NCOL], f32, tag="R2")
                    nc.vector.memset(R2[0:1, :], 1.0)
                    nc.sync.dma_start(out=R2[1:2, :], in_=as_sb[:, :])

                    hpA, hpB = [], []
                    for b in range(NB):
                        pn = gsq.tile([128, 128], f32, tag="sq")
                        nc.tensor.matmul(pn[:, :], lhsT=nfT[:, b * N:b * N + 128],
                                         rhs=gW, start=True, stop=True)
                        ha = gw.tile([128, H], f32, name=f"hpA{b}", tag=f"hpA{b}")
                        nc.vector.tensor_copy(ha[:, :], pn[:, :])
                        hpA.append(ha)
                        pn2 = gsq.tile([2, 128], f32, tag="tiny")
                        nc.tensor.matmul(pn2[:, :], lhsT=nfT[:, b * N + 128:b * N + N],
                                         rhs=gW, start=True, stop=True)
                        hb = gw.tile([2, H], f32, name=f"hpB{b}", tag=f"hpB{b}")
                        nc.vector.tensor_copy(hb[:, :], pn2[:, :])
                        hpB.append(hb)

                    nfT_next = gw.tile([H, NCOL], f32, tag="nfT")
                    for b in range(NB):
                        L2b = gw.tile([2, N], f32, tag="L2b")
                        nc.vector.tensor_copy(L2b[0:1, :], ad_sb[0:1, b * N:(b + 1) * N])
                        nc.sync.dma_start(out=L2b[1:2, :], in_=ones_row[0:1, 0:N])

                        pa = gsq.tile([128, N], f32, tag="sq")
                        nc.tensor.matmul(pa[:, :N], lhsT=L2b[:, 0:128],
                                         rhs=R2[:, b * N:(b + 1) * N],
                                         start=True, stop=True)
                        pb = gsq.tile([2, N], f32, tag="tiny")
                        nc.tensor.matmul(pb[:, :N], lhsT=L2b[:, 128:N],
                                         rhs=R2[:, b * N:(b + 1) * N],
                                         start=True, stop=True)

                        PA = gw.tile([128, N], f32, tag="PA")
                        PB = gw.tile([2, N], f32, tag="PB")
                        sA_ = gw.tile([128, 1], f32, tag="sA")
                        sB_ = gw.tile([2, 1], f32, tag="sB")
                        for (pp, ew, Ct, Pt, st, rows) in (
                                (pa, ewA[b], CAt, PA, sA_, 128),
                                (pb, ewB[b], CBt, PB, sB_, 2)):
                            t_ = gw.tile([rows, N], f32, tag=f"t{rows}")
                            nc.scalar.activation(t_[:, :], pp[:rows, :N], AF.Prelu,
                                                 alpha=alpha02[:rows, :])
                            z_ = gw.tile([rows, N], f32, tag=f"z{rows}")
                            nc.vector.tensor_tensor(out=z_[:, :], in0=t_[:, :],
                                                    in1=ew[:, :], op=OP.mult)
                            e_ = gw.tile([rows, N], f32, tag=f"e{rows}")
                            nc.scalar.activation(e_[:, :], z_[:, :], AF.Exp)
                            nc.vector.scalar_tensor_tensor(
                                out=Pt[:, :], in0=e_[:, :], scalar=1.0,
                                in1=Ct[:, :], op0=OP.mult, op1=OP.mult,
                                accum_out=st[:, :])

                        rA = gw.tile([128, 1], f32, tag="rA")
                        rAn = gw.tile([128, 1], f32, tag="rAn")
                        rB = gw.tile([2, 1], f32, tag="rB")
                        rBn = gw.tile([2, 1], f32, tag="rBn")
                        for st, rr, rn in ((sA_, rA, rAn), (sB_, rB, rBn)):
                            nc.vector.tensor_scalar_add(st[:, :], st[:, :], 1e-8)
                            nc.vector.reciprocal(rr[:, :], st[:, :])
                            nc.vector.tensor_scalar_mul(rn[:, :], rr[:, :], -1.0)

                        # transpose P -> PT (src-major) for the aggregation
                        PT = gw.tile([128, N], f32, tag="PT")
                        PT2 = gw.tile([2, N], f32, tag="PT2")
                        pt1 = gsq.tile([128, 128], f32, tag="sq")
                        nc.tensor.transpose(pt1[:, :], PA[:, 0:128], ident[:, :])
                        nc.vector.tensor_copy(PT[:, 0:128], pt1[:, :])
                        pt2 = gsq.tile([2, 128], f32, tag="tiny")
                        nc.tensor.transpose(pt2[:, :], PA[:, 128:N], ident[:, :])
                        nc.vector.tensor_copy(PT2[:, 0:128], pt2[:, :])
                        pt3 = gsq.tile([128, 2], f32, tag="col2", bufs=1)
                        nc.tensor.transpose(pt3[:, :], PB[:, 0:128], ident[0:2, 0:2])
                        nc.vector.tensor_copy(PT[:, 128:N], pt3[:, :])
                        pt4 = gsq.tile([2, 2], f32, tag="tiny")
                        nc.tensor.transpose(pt4[:, :], PB[:, 128:N], ident[0:2, 0:2])
                        nc.vector.tensor_copy(PT2[:, 128:N], pt4[:, :])

                        po = gsq.tile([128, H], f32, tag="sq")
                        nc.tensor.matmul(po[:, :], lhsT=PT[:, 0:128], rhs=hpA[b][:, :],
                                         start=True, stop=False)
                        nc.tensor.matmul(po[:, :], lhsT=PT2[:, 0:128], rhs=hpB[b][:, :],
                                         start=False, stop=True)
                        po2 = gsq.tile([2, H], f32, tag="tiny")
                        nc.tensor.matmul(po2[:, :], lhsT=PT[:, 128:N], rhs=hpA[b][:, :],
                                         start=True, stop=False)
                        nc.tensor.matmul(po2[:, :], lhsT=PT2[:, 128:N], rhs=hpB[b][:, :],
                                         start=False, stop=True)

                        # elu(out * r) eviction, then transpose back to feat-major
                        for (pp, rr, rn, rows, coff) in (
                                (po, rA, rAn, 128, 0), (po2, rB, rBn, 2, 128)):
                            pos = gw.tile([rows, H], f32, tag=f"pos{rows}")
                            nc.scalar.activation(pos[:, :], pp[:rows, :], AF.Relu,
                                                 scale=rr[:rows, :])
                            neg = gw.tile([rows, H], f32, tag=f"neg{rows}")
                            nc.scalar.activation(neg[:, :], pp[:rows, :], AF.Relu,
                                                 scale=rn[:rows, :])
                            ex = gw.tile([rows, H], f32, tag=f"ex{rows}")
                            nc.scalar.activation(ex[:, :], neg[:, :], AF.Exp,
                                                 scale=-1.0)
                            nf_ = gw.tile([rows, H], f32, tag=f"nf{rows}")
                            nc.vector.scalar_tensor_tensor(
                                out=nf_[:, :], in0=ex[:, :], scalar=1.0,
                                in1=pos[:, :], op0=OP.subtract, op1=OP.add)
                            if rows == 128:
                                ptb = gsq.tile([128, 128], f32, tag="sq")
                                nc.tensor.transpose(ptb[:, :], nf_[:, :], ident[:, :])
                                nc.vector.tensor_copy(
                                    nfT_next[:, b * N:b * N + 128], ptb[:, :])
                            else:
                                ptb = gsq.tile([128, 2], f32, tag="col2", bufs=1)
                                nc.tensor.transpose(ptb[:, :], nf_[:, :],
                                                    ident[0:2, 0:2])
                                nc.vector.tensor_copy(
                                    nfT_next[:, b * N + 128:b * N + N], ptb[:, :])
                    nfT = nfT_next

            # ---------------- stage D: per-asset heads + softmax ------------
            with ExitStack() as sD:
                hw = sD.enter_context(tc.tile_pool(name="hw", bufs=1))
                hps = sD.enter_context(tc.tile_pool(name="hps", bufs=1, space="PSUM"))
                hsq = sD.enter_context(tc.tile_pool(name="hsq", bufs=4, space="PSUM"))

                hid_ps = hps.tile([64, A * NB], f32, tag="hid")
                for a in range(A):
                    rhs = bass.AP(tensor=nfT.tensor, offset=nfT.offset + a,
                                  ap=[list(nfT.ap[0]), [N, NB]])
                    nc.tensor.matmul(hid_ps[:, a * NB:(a + 1) * NB],
                                     lhsT=hW1T[:, a * 64:(a + 1) * 64],
                                     rhs=rhs, start=True, stop=True)
                hid = hw.tile([64, A * NB], f32)
                nc.vector.tensor_tensor(out=hid[:, :], in0=hid_ps[:, :],
                                        in1=b1exp[:, :], op=OP.add)
                nc.scalar.activation(hid[:, :], hid[:, :], AF.Relu)

                log_ps = hps.tile([ODIM, A * NB], f32, tag="log")
                for a in range(A):
                    nc.tensor.matmul(log_ps[:, a * NB:(a + 1) * NB],
                                     lhsT=hW2T[:, a * ODIM:(a + 1) * ODIM],
                                     rhs=hid[:, a * NB:(a + 1) * NB],
                                     start=True, stop=True)
                logits = hw.tile([ODIM, A * NB], f32)
                nc.vector.tensor_tensor(out=logits[:, :], in0=log_ps[:, :],
                                        in1=b2exp[:, :], op=OP.add)
                nc.sync.dma_start(out=o_logits[:, :], in_=logits[:, :])

                # softmax over ODIM: transpose to (128, 4, 3), exp on eviction
                e_sb = hw.tile([128, NB * ODIM], f32)
                for c in range(NB):
                    pt = hsq.tile([128, ODIM], f32, tag="sm")
                    nc.tensor.transpose(pt[:, :], logits[:, c * 128:(c + 1) * 128],
                                        ident[0:ODIM, 0:ODIM])
                    nc.scalar.activation(e_sb[:, c * ODIM:(c + 1) * ODIM],
                                         pt[:, :], AF.Exp)
                s_sb = hw.tile([128, NB], f32)
                for c in range(NB):
                    nc.vector.tensor_tensor(out=s_sb[:, c:c + 1],
                                            in0=e_sb[:, c * ODIM:c * ODIM + 1],
                                            in1=e_sb[:, c * ODIM + 1:c * ODIM + 2],
                                            op=OP.add)
                    nc.vector.tensor_tensor(out=s_sb[:, c:c + 1],
                                            in0=s_sb[:, c:c + 1],
                                            in1=e_sb[:, c * ODIM + 2:c * ODIM + 3],
                                            op=OP.add)
                r_sb = hw.tile([128, NB], f32)
                nc.vector.reciprocal(r_sb[:, :], s_sb[:, :])
                probs = hw.tile([128, NB * ODIM], f32)
                r_b = _bcast_ap(r_sb, 0, [[1, NB], [0, ODIM]])
                nc.vector.tensor_tensor(out=probs[:, :], in0=e_sb[:, :],
                                        in1=r_b, op=OP.mult)
                nc.sync.dma_start(out=o_probs[:, :], in_=probs[:, :])

    return nc


def host_inputs(x, edge_index, W_emb, b_emb, conv_w, conv_b, bn_gamma, bn_beta,
                bn_mean, bn_var, gat_W, gat_a_src, gat_a_dst, ew_W1, ew_b1,
                ew_W2, ew_b2, head_W1, head_b1, head_W2, head_b2):
    """Per-core input dicts (host-side preprocessing)."""
    f = np.float32
    xs = np.asarray(x, f)[:, :, T - W:, :]                       # (B,N,15,64)
    xt = np.ascontiguousarray(np.transpose(xs, (3, 0, 1, 2)))    # (64,B,N,15)

    ei = np.asarray(edge_index)
    C = np.zeros((N, N), f)
    np.add.at(C, (ei[1].astype(np.int64), ei[0].astype(np.int64)), 1.0)

    inv = np.asarray(bn_gamma, f) / np.sqrt(np.asarray(bn_var, f) + BN_EPS)
    sc_all = inv.T.copy()                                        # (H,3)
    bi_all = ((np.asarray(conv_b, f) - np.asarray(bn_mean, f)) * inv
              + np.asarray(bn_beta, f)).T.copy()                 # (H,3)
    cw = np.asarray(conv_w, f)                                   # (3,H,H,3)
    cw_all = np.concatenate(
        [cw[i, :, :, k].T for i in range(3) for k in range(3)], axis=1)

    ew_W1 = np.asarray(ew_W1, f)
    gat_W = np.asarray(gat_W, f)
    hW1 = np.asarray(head_W1, f); hW2 = np.asarray(head_W2, f)
    # b1exp[k, a*NB+bi] = head_b1[a,k]
    b1exp = np.repeat(np.asarray(head_b1, f).T[:, :, None], NB, axis=2)
    b1exp = b1exp.reshape(64, A * NB)
    b2exp = np.repeat(np.asarray(head_b2, f).T[:, :, None], NB, axis=2)
    b2exp = b2exp.reshape(ODIM, A * NB)

    shared = {
        "W_embT": np.ascontiguousarray(np.asarray(W_emb, f).T),
        "b_emb": np.asarray(b_emb, f).reshape(H, 1),
        "cw_all": np.ascontiguousarray(cw_all),
        "sc_all": np.ascontiguousarray(sc_all),
        "bi_all": np.ascontiguousarray(bi_all),
        "W1aT": np.ascontiguousarray(ew_W1[:, :H].T),
        "W1bT": np.ascontiguousarray(ew_W1[:, H:].T),
        "b1": np.asarray(ew_b1, f).reshape(H, 1),
        "w2": np.ascontiguousarray(np.asarray(ew_W2, f).reshape(1, H).T),
        "b2ew": np.asarray(ew_b2, f).reshape(1, 1),
        "CA": np.ascontiguousarray(C[:128]),
        "CB": np.ascontiguousarray(C[128:]),
        "gWT": np.ascontiguousarray(
            np.concatenate([gat_W[i].T for i in range(3)], axis=1)),
        "asrc": np.ascontiguousarray(
            np.stack([np.asarray(gat_a_src, f)[i, 0] for i in range(3)], axis=1)),
        "adst": np.ascontiguousarray(
            np.stack([np.asarray(gat_a_dst, f)[i, 0] for i in range(3)], axis=1)),
        "hW1T": np.ascontiguousarray(
            np.concatenate([hW1[a].T for a in range(A)], axis=1)),
        "b1exp": np.ascontiguousarray(b1exp),
        "hW2T": np.ascontiguousarray(
            np.concatenate([hW2[a].T for a in range(A)], axis=1)),
        "b2exp": np.ascontiguousarray(b2exp),
    }
    in_maps = []
    for c in range(NC_CORES):
        m = dict(shared)
        m["xt"] = np.ascontiguousarray(
            xt[:, c * NB:(c + 1) * NB].reshape(DIN, NCOL * W))
        in_maps.append(m)
    return in_maps


_CACHE = {}


def kernel(**inputs):
    _apply_sync_split_patch()
    if "nc" not in _CACHE:
        _CACHE["nc"] = build_program()
    nc = _CACHE["nc"]
    in_maps = host_inputs(**inputs)
    res = run_bass_kernel_spmd(nc, in_maps, list(range(NC_CORES)), trace=False)
    logits = np.empty((B, A, ODIM), np.float32)
    probs = np.empty((B, A, ODIM), np.float32)
    for c in range(NC_CORES):
        lg = res.results[c]["logits"]          # (3, A*NB)
        pr = res.results[c]["probs"]           # (128, NB*3)
        logits[c * NB:(c + 1) * NB] = lg.reshape(ODIM, A, NB).transpose(2, 1, 0)
        # probs rows: chunk c2 covers logit cols c2*128..; col idx = a*NB+bi
        tmp = pr.reshape(128, NB, ODIM).transpose(1, 0, 2).reshape(A * NB, ODIM)
        probs[c * NB:(c + 1) * NB] = tmp.reshape(A, NB, ODIM).transpose(1, 0, 2)
    return logits, probs



# revision 8
# speedup vs baseline: 2.7889x; 2.7889x over previous
"""CrossAssetGNN forward on 8 Trainium2 cores, data-parallel over batch.

v2 — bf16 matmuls + restructured stages vs the v1 baseline:
- Embedding folded into conv1 host-side (consecutive linear ops); convs
  evaluate only the 7/3/1 tap positions the last-timestep readout needs.
- Dense edge-MLP contraction streams relu(U_i+V_j) through a stationary
  w2 (padded to M=32) with 4-way PE column-tiling (one column-group per
  graph); bank-aligned 512-col sub-matmuls let one contiguous ACT apply
  the sigmoid on eviction; a reshape DMA transposes each graph's row to
  the [src, dst] tile the GAT layers consume.
- GAT attention is computed directly in [src, dst] layout so no P-matrix
  transposes are needed; softmax denominators ride the aggregation
  matmul as an appended ones-column.
- Per-asset heads pack 2 assets per matmul (128-wide stationary) and the
  second layer packs 4 assets per round via row+col tile_position.
"""
import json
import sys

sys.path.insert(0, "/opt/trn_rl_repo")

import numpy as np
from ml_dtypes import bfloat16
from contextlib import ExitStack

import concourse.bass as bass
import concourse.tile as tile
from concourse import masks, mybir
from concourse.bass_utils import run_bass_kernel_spmd

f32 = mybir.dt.float32
bf16 = mybir.dt.bfloat16
AF = mybir.ActivationFunctionType
OP = mybir.AluOpType

B, A, AUX, T, DIN, H, ODIM = 32, 128, 2, 128, 64, 128, 3
N = A + AUX            # 130
NC_CORES = 8
NB = B // NC_CORES     # 4 graphs per core
W = 15                 # receptive field of the three causal convs
BN_EPS = 1e-5
NCOL = NB * N          # 520 node columns per core


# ---- walrus workaround: max 1 sync-wait command per instruction ------------
def _apply_sync_split_patch():
    if getattr(bass.Bass, "_sync_split_patched", False):
        return
    orig = bass.Bass.to_json_bytes

    def to_json_bytes(self, *a, **kw):
        m = json.loads(orig(self, *a, **kw))
        for f in m.get("functions", []):
            for blk in f.get("blocks", []):
                new = []
                for inst in blk.get("instructions", []):
                    si = inst.get("sync_info")
                    if (si and si.get("on_wait") and len(si["on_wait"]) > 1
                            and inst.get("engine") in
                            {"PE", "DVE", "Activation", "SP", "Pool"}):
                        waits = si["on_wait"]
                        for k, w in enumerate(waits[:-1]):
                            new.append({"engine": inst["engine"], "ins": [],
                                        "outs": [],
                                        "name": f"{inst['name']}-sw{k}",
                                        "opcode": "NoOp",
                                        "sync_info": {"on_update": [],
                                                      "on_wait": [w]}})
                        si["on_wait"] = waits[-1:]
                    new.append(inst)
                blk["instructions"] = new
        return json.dumps(m).encode()

    bass.Bass.to_json_bytes = to_json_bytes
    bass.Bass._sync_split_patched = True


def _ap(t, offset_elems, dims):
    """Raw AP over tile t: keep t's partition dim, custom free dims."""
    return bass.AP(tensor=t.tensor, offset=t.offset + offset_elems,
                   ap=[list(t.ap[0])] + [list(d) for d in dims])


def build_program():
    nc = bass.Bass("TRN2", target_bir_lowering=False, num_devices=NC_CORES)

    din = {}

    def d_in(name, shape, dt=f32):
        din[name] = nc.dram_tensor(name, list(shape), dt, kind="ExternalInput")
        return din[name]

    d_in("xt", [DIN, NCOL * W], bf16)
    d_in("cw1", [DIN, 3 * H], bf16)
    d_in("cw23", [H, 6 * H], bf16)
    d_in("sc_all", [H, 3]); d_in("bi_all", [H, 3])
    d_in("W1aT", [H, H], bf16); d_in("W1bT", [H, H], bf16)
    d_in("b1", [H, 1]); d_in("w2pad", [H, 32], bf16); d_in("b2ew", [1, 1])
    d_in("CTA", [128, N]); d_in("CTB", [2, N])
    d_in("gWT", [H, 3 * H], bf16)
    d_in("ascol", [H, 3], bf16); d_in("adcol", [H, 3], bf16)
    d_in("hW1T", [H, A * 64], bf16)
    d_in("b1p", [128, 512])
    d_in("hW2Td", [128, A * ODIM], bf16)
    d_in("b2p", [128, 128])

    o_logits = nc.dram_tensor("logits", [128, 128], f32, kind="ExternalOutput")
    o_probs = nc.dram_tensor("probs", [128, NB * ODIM], f32,
                             kind="ExternalOutput")

    with tile.TileContext(nc) as tc:
        with ExitStack() as top:
            const = top.enter_context(tc.tile_pool(name="const", bufs=1))
            persist = top.enter_context(tc.tile_pool(name="persist", bufs=1))

            def load(name, shape, dt=f32):
                t = const.tile(list(shape), dt, name=f"c_{name}",
                               tag=f"c_{name}")
                nc.sync.dma_start(out=t, in_=din[name][:, :])
                return t

            cw1 = load("cw1", [DIN, 3 * H], bf16)
            sc_all = load("sc_all", [H, 3]); bi_all = load("bi_all", [H, 3])
            cw23 = load("cw23", [H, 6 * H], bf16)
            W1aT = load("W1aT", [H, H], bf16)
            W1bT = load("W1bT", [H, H], bf16)
            b1 = load("b1", [H, 1])
            w2pad = load("w2pad", [H, 32], bf16)
            CTA = load("CTA", [128, N]); CTB = load("CTB", [2, N])
            gWT = load("gWT", [H, 3 * H], bf16)
            ascol = load("ascol", [H, 3], bf16)
            adcol = load("adcol", [H, 3], bf16)

            b2ap = din["b2ew"][:, :]
            b2col = const.tile([128, 1], f32)
            nc.sync.dma_start(out=b2col, in_=bass.AP(
                tensor=b2ap.tensor, offset=b2ap.offset, ap=[[0, 128], [1, 1]]))

            identF = const.tile([128, 128], f32)
            masks.make_identity(nc, identF[:, :])
            identB = const.tile([128, 128], bf16)
            masks.make_identity(nc, identB[:, :])
            alpha02 = const.tile([128, 1], f32)
            nc.vector.memset(alpha02[:, :], 0.2)
            ones_bf = const.tile([1, NCOL], bf16)
            nc.vector.memset(ones_bf[:, :], 1.0)

            hW1T = load("hW1T", [H, A * 64], bf16)
            b1p = load("b1p", [128, 512])
            hW2Td = load("hW2Td", [128, A * ODIM], bf16)
            b2p = load("b2p", [128, 128])

            feats = persist.tile([H, NCOL], bf16)
            ewTA = persist.tile([128, NCOL], f32)
            ewTB = persist.tile([2, NCOL], f32)

            # ======== stage A: folded embed + 3 dilated causal convs ======
            with ExitStack() as sA:
                front = sA.enter_context(tc.tile_pool(name="front", bufs=1))
                psA = sA.enter_context(
                    tc.tile_pool(name="psA", bufs=3, space="PSUM"))

                for g in range(NB):
                    xg = front.tile([DIN, N * W], bf16, name=f"xg{g}",
                                    tag=f"xg{g}")
                    nc.sync.dma_start(
                        out=xg, in_=din["xt"][:, g * N * W:(g + 1) * N * W])

                    l1 = front.tile([H, N * 7], bf16, name=f"l1_{g}",
                                    tag=f"l1_{g}")
                    for b0, nb in ((0, 72), (72, 58)):
                        pe = psA.tile([128, 504], f32, tag="pe")
                        for k in range(3):
                            rhs = _ap(xg, b0 * W + k, [[W, nb], [2, 7]])
                            nc.tensor.matmul(pe[:, :nb * 7],
                                             lhsT=cw1[:, k * H:(k + 1) * H],
                                             rhs=rhs, start=(k == 0),
                                             stop=(k == 2))
                        nc.scalar.activation(
                            l1[:, b0 * 7:(b0 + nb) * 7], pe[:, :nb * 7],
                            AF.Gelu, bias=bi_all[:, 0:1], scale=sc_all[:, 0:1])

                    l2 = front.tile([H, N * 3], bf16, name=f"l2_{g}",
                                    tag=f"l2_{g}")
                    pe = psA.tile([128, 504], f32, tag="pe")
                    for k in range(3):
                        rhs = _ap(l1, k, [[7, N], [2, 3]])
                        nc.tensor.matmul(pe[:, :N * 3],
                                         lhsT=cw23[:, k * H:(k + 1) * H],
                                         rhs=rhs, start=(k == 0), stop=(k == 2))
                    nc.scalar.activation(l2[:, :], pe[:, :N * 3], AF.Gelu,
                                         bias=bi_all[:, 1:2],
                                         scale=sc_all[:, 1:2])

                    pe = psA.tile([128, 504], f32, tag="pe")
                    for k in range(3):
                        rhs = _ap(l2, k, [[3, N], [1, 1]])
                        nc.tensor.matmul(pe[:, :N],
                                         lhsT=cw23[:, (3 + k) * H:(4 + k) * H],
                                         rhs=rhs, start=(k == 0), stop=(k == 2))
                    nc.scalar.activation(feats[:, g * N:(g + 1) * N],
                                         pe[:, :N], AF.Gelu,
                                         bias=bi_all[:, 2:3],
                                         scale=sc_all[:, 2:3])

            # ======== stage B: dense edge-weight MLP ======================
            with ExitStack() as sB:
                ewk = sB.enter_context(tc.tile_pool(name="ewk", bufs=2))
                big = sB.enter_context(tc.tile_pool(name="ewbig", bufs=1))

                Ut = big.tile([H, NCOL], bf16)
                Vt = big.tile([H, NCOL], bf16)
                with ExitStack() as sB1:
                    psU = sB1.enter_context(
                        tc.tile_pool(name="psU", bufs=2, space="PSUM"))
                    for s in range(0, NCOL, 260):
                        pu = psU.tile([128, 260], f32, tag="uv")
                        nc.tensor.matmul(pu[:, :], lhsT=W1aT[:, :],
                                         rhs=feats[:, s:s + 260], start=True,
                                         stop=True)
                        nc.vector.tensor_copy(Ut[:, s:s + 260], pu[:, :])
                        pv = psU.tile([128, 260], f32, tag="uv")
                        nc.tensor.matmul(pv[:, :], lhsT=W1bT[:, :],
                                         rhs=feats[:, s:s + 260], start=True,
                                         stop=True)
                        nc.scalar.activation(Vt[:, s:s + 260], pv[:, :],
                                             AF.Identity, bias=b1[:, :])

                ew_rows = big.tile([128, N * N], f32)

                with ExitStack() as sB2:
                    psE = sB2.enter_context(
                        tc.tile_pool(name="psE", bufs=2, space="PSUM"))
                    for t in range(11):
                        js = 12 if t < 10 else 10
                        cols = js * N
                        Rg = []
                        for g in range(NB):
                            R = ewk.tile([128, 12 * N], bf16, name=f"R{g}",
                                         tag=f"R{g}")
                            in0 = _ap(Ut, g * N + t * 12, [[1, js], [0, N]])
                            in1 = _ap(Vt, g * N, [[0, js], [1, N]])
                            nc.vector.tensor_tensor(out=R[:, :cols], in0=in0,
                                                    in1=in1, op=OP.add)
                            if g < 3:
                                nc.scalar.activation(R[:, :cols], R[:, :cols],
                                                     AF.Relu)
                            else:
                                nc.vector.tensor_scalar_max(R[:, :cols],
                                                            R[:, :cols], 0.0)
                            Rg.append(R)
                        pew = psE.tile([128, 2048], f32, tag="ew")
                        for g in range(NB):
                            for s0 in range(0, cols, 512):
                                sl = min(512, cols - s0)
                                nc.tensor.matmul(
                                    pew[32 * g:32 * g + 32, s0:s0 + sl],
                                    lhsT=w2pad[:, :],
                                    rhs=Rg[g][:, s0:s0 + sl],
                                    start=True, stop=True,
                                    tile_position=(0, 32 * g))
                        nc.scalar.activation(
                            ew_rows[:, t * 12 * N:t * 12 * N + cols],
                            pew[:, :cols], AF.Sigmoid, bias=b2col[:, :])

                # reshape each graph's row to [src, dst] tiles
                for g in range(NB):
                    sl = ew_rows[32 * g:32 * g + 1, 0:1]
                    nc.sync.dma_start(
                        out=ewTA[:, g * N:(g + 1) * N],
                        in_=bass.AP(tensor=sl.tensor, offset=sl.offset,
                                    ap=[list(sl.ap[0]), [N, 128], [1, N]]))
                    nc.sync.dma_start(
                        out=ewTB[:, g * N:(g + 1) * N],
                        in_=bass.AP(tensor=sl.tensor,
                                    offset=sl.offset + 128 * N,
                                    ap=[list(sl.ap[0]), [N, 2], [1, N]]))

            # ======== stage C: 3 dense GAT layers =========================
            nfT = feats
            with ExitStack() as sC:
                gw = sC.enter_context(tc.tile_pool(name="gw", bufs=2))
                gps = sC.enter_context(
                    tc.tile_pool(name="gps", bufs=3, space="PSUM"))
                gtiny = sC.enter_context(
                    tc.tile_pool(name="gtiny", bufs=2, space="PSUM"))
                grow = sC.enter_context(
                    tc.tile_pool(name="grow", bufs=2, space="PSUM"))

                for li in range(3):
                    gW = gWT[:, li * H:(li + 1) * H]
                    last = li == 2

                    # hpT (feat-major) for as/ad rows
                    hpT = gw.tile([H, NCOL], bf16, tag="hpT")
                    for s in range(0, NCOL, 260):
                        ph = gps.tile([128, 260], f32, tag="sq")
                        nc.tensor.matmul(ph[:, :], lhsT=gW,
                                         rhs=nfT[:, s:s + 260], start=True,
                                         stop=True)
                        nc.vector.tensor_copy(hpT[:, s:s + 260], ph[:, :])

                    # as/ad rows packed with ones rows for K=2 outer-sum
                    asL = gw.tile([2, NCOL], bf16, tag="asL")
                    adR = gw.tile([2, NCOL], bf16, tag="adR")
                    ad_sb = gw.tile([1, NCOL], bf16, tag="ad_sb")
                    nc.sync.dma_start(out=asL[1:2, :], in_=ones_bf[:, :])
                    nc.vector.memset(adR[0:1, :], 1.0)
                    for s in range(0, NCOL, 260):
                        pr = grow.tile([1, 260], f32, tag="row")
                        nc.tensor.matmul(pr[:, :], lhsT=ascol[:, li:li + 1],
                                         rhs=hpT[:, s:s + 260], start=True,
                                         stop=True)
                        nc.scalar.copy(asL[0:1, s:s + 260], pr[:, :])
                        pr2 = grow.tile([1, 260], f32, tag="row")
                        nc.tensor.matmul(pr2[:, :], lhsT=adcol[:, li:li + 1],
                                         rhs=hpT[:, s:s + 260], start=True,
                                         stop=True)
                        nc.scalar.copy(ad_sb[0:1, s:s + 260], pr2[:, :])
                    nc.sync.dma_start(out=adR[1:2, :], in_=ad_sb[:, :])

                    # node-major hp (+ ones col) per graph
                    hpA, hpB = [], []
                    for g in range(NB):
                        ha = gw.tile([128, H + 1], bf16, name=f"hpA{g}",
                                     tag=f"hpA{g}")
                        ph = gps.tile([128, 260], f32, tag="sq")
                        nc.tensor.matmul(ph[:, :H],
                                         lhsT=nfT[:, g * N:g * N + 128],
                                         rhs=gW, start=True, stop=True)
                        if g % 2 == 0:
                            nc.vector.tensor_copy(ha[:, 0:H], ph[:, :H])
                        else:
                            nc.scalar.copy(ha[:, 0:H], ph[:, :H])
                        nc.vector.memset(ha[:, H:H + 1], 1.0)
                        hpA.append(ha)
                        hb = gw.tile([2, H + 1], bf16, name=f"hpB{g}",
                                     tag=f"hpB{g}")
                        ph2 = gtiny.tile([2, 260], f32, tag="tiny")
                        nc.tensor.matmul(ph2[:, :H],
                                         lhsT=nfT[:, g * N + 128:g * N + N],
                                         rhs=gW, start=True, stop=True)
                        nc.scalar.copy(hb[:, 0:H], ph2[:, :H])
                        nc.vector.memset(hb[:, H:H + 1], 1.0)
                        hpB.append(hb)

                    # attention logits pa[s,d] = as[s] + ad[d], prelu evict
                    tA = gw.tile([128, NCOL], f32, tag="tA")
                    tB = gw.tile([2, NCOL], f32, tag="tB")
                    for g in range(NB):
                        pa = gps.tile([128, 260], f32, tag="sq")
                        nc.tensor.matmul(pa[:, :N],
                                         lhsT=asL[:, g * N:g * N + 128],
                                         rhs=adR[:, g * N:(g + 1) * N],
                                         start=True, stop=True)
                        nc.scalar.activation(tA[:, g * N:(g + 1) * N],
                                             pa[:, :N], AF.Prelu,
                                             alpha=alpha02[:, :])
                        pb = gtiny.tile([2, 260], f32, tag="tiny")
                        nc.tensor.matmul(pb[:, :N],
                                         lhsT=asL[:, g * N + 128:g * N + N],
                                         rhs=adR[:, g * N:(g + 1) * N],
                                         start=True, stop=True)
                        nc.scalar.activation(tB[:, g * N:(g + 1) * N],
                                             pb[:, :N], AF.Prelu,
                                             alpha=alpha02[0:2, :])

                    # z = t*ewT ; P~T = exp(z)*CT  (bf16 out for matmul)
                    nc.vector.tensor_tensor(out=tA[:, :], in0=tA[:, :],
                                            in1=ewTA[:, :], op=OP.mult)
                    nc.scalar.activation(tA[:, :], tA[:, :], AF.Exp)
                    PTA = gw.tile([128, NCOL], bf16, tag="PTA")
                    nc.vector.tensor_tensor(
                        out=PTA[:, :], in0=tA[:, :],
                        in1=_ap(CTA, 0, [[0, NB], [1, N]]), op=OP.mult)
                    nc.vector.tensor_tensor(out=tB[:, :], in0=tB[:, :],
                                            in1=ewTB[:, :], op=OP.mult)
                    nc.scalar.activation(tB[:, :], tB[:, :], AF.Exp)
                    PTB = gw.tile([2, NCOL], bf16, tag="PTB")
                    nc.vector.tensor_tensor(
                        out=PTB[:, :], in0=tB[:, :],
                        in1=_ap(CTB, 0, [[0, NB], [1, N]]), op=OP.mult)

                    # aggregation + elu eviction + transpose back
                    nfT_next = gw.tile([H, NCOL], bf16, tag="nfT")
                    for g in range(NB):
                        po = gps.tile([128, 260], f32, tag="sq")
                        nc.tensor.matmul(po[:, :H + 1],
                                         lhsT=PTA[:, g * N:g * N + 128],
                                         rhs=hpA[g][:, :], start=True,
                                         stop=False)
                        nc.tensor.matmul(po[:, :H + 1],
                                         lhsT=PTB[:, g * N:g * N + 128],
                                         rhs=hpB[g][:, :], start=False,
                                         stop=True)
                        parts = [(po, 128, 0)]
                        if not last:
                            po2 = gtiny.tile([2, 260], f32, tag="tiny")
                            nc.tensor.matmul(po2[:, :H + 1],
                                             lhsT=PTA[:, g * N + 128:g * N + N],
                                             rhs=hpA[g][:, :], start=True,
                                             stop=False)
                            nc.tensor.matmul(po2[:, :H + 1],
                                             lhsT=PTB[:, g * N + 128:g * N + N],
                                             rhs=hpB[g][:, :], start=False,
                                             stop=True)
                            parts.append((po2, 2, 128))

                        for pp, rows, coff in parts:
                            rr = gw.tile([rows, 1], f32, tag=f"rr{rows}")
                            rn = gw.tile([rows, 1], f32, tag=f"rn{rows}")
                            nc.vector.tensor_scalar_add(rr[:, :],
                                                        pp[:rows, H:H + 1],
                                                        1e-8)
                            nc.vector.reciprocal(rr[:, :], rr[:, :])
                            nc.vector.tensor_scalar_mul(rn[:, :], rr[:, :],
                                                        -1.0)
                            pos = gw.tile([rows, H], f32, tag=f"pos{rows}")
                            nc.scalar.activation(pos[:, :], pp[:rows, 0:H],
                                                 AF.Relu, scale=rr[:, :])
                            neg = gw.tile([rows, H], f32, tag=f"neg{rows}")
                            nc.scalar.activation(neg[:, :], pp[:rows, 0:H],
                                                 AF.Relu, scale=rn[:, :])
                            ex = gw.tile([rows, H], f32, tag=f"ex{rows}")
                            nc.scalar.activation(ex[:, :], neg[:, :], AF.Exp,
                                                 scale=-1.0)
                            nfg = gw.tile([rows, H], bf16, tag=f"nfg{rows}")
                            nc.vector.scalar_tensor_tensor(
                                out=nfg[:, :], in0=ex[:, :], scalar=1.0,
                                in1=pos[:, :], op0=OP.subtract, op1=OP.add)
                            if rows == 128:
                                pt = gps.tile([128, 260], bf16, tag="sq")
                                nc.tensor.transpose(pt[:, :128], nfg[:, :],
                                                    identB[:, :])
                                nc.vector.tensor_copy(
                                    nfT_next[:, g * N:g * N + 128],
                                    pt[:, :128])
                            else:
                                pt = gps.tile([128, 260], bf16, tag="sq")
                                nc.tensor.transpose(pt[:, 0:2], nfg[:, :],
                                                    identB[0:2, 0:2])
                                nc.vector.tensor_copy(
                                    nfT_next[:, g * N + 128:g * N + N],
                                    pt[:, 0:2])
                    nfT = nfT_next

            # ======== stage D: per-asset heads + softmax ==================
            with ExitStack() as sD:
                hw = sD.enter_context(tc.tile_pool(name="hw", bufs=1))
                hps = sD.enter_context(
                    tc.tile_pool(name="hps", bufs=1, space="PSUM"))

                hid_ps = hps.tile([128, 512], f32, tag="hid")
                for p in range(64):
                    rhs = bass.AP(tensor=nfT.tensor,
                                  offset=nfT.offset + 2 * p,
                                  ap=[list(nfT.ap[0]), [1, 2], [N, NB]])
                    nc.tensor.matmul(hid_ps[:, p * 8:(p + 1) * 8],
                                     lhsT=hW1T[:, p * 128:(p + 1) * 128],
                                     rhs=rhs, start=True, stop=True)
                hidb = hw.tile([128, 512], f32)
                nc.vector.tensor_tensor(out=hidb[:, :], in0=hid_ps[:, :],
                                        in1=b1p[:, :], op=OP.add)
                hid_sb = hw.tile([128, 512], bf16)
                nc.scalar.activation(hid_sb[:, :], hidb[:, :], AF.Relu)

                log_ps = hps.tile([128, 128], f32, tag="log")
                for q in range(32):
                    for j in range(4):
                        a = 4 * q + j
                        par = a & 1
                        nc.tensor.matmul(
                            log_ps[32 * j:32 * j + 3, 4 * q:4 * q + 4],
                            lhsT=hW2Td[64 * par:64 * par + 64,
                                       a * 3:(a + 1) * 3],
                            rhs=hid_sb[64 * par:64 * par + 64,
                                       (a >> 1) * 8 + 4 * par:
                                       (a >> 1) * 8 + 4 * par + 4],
                            start=True, stop=True,
                            tile_position=(64 * par, 32 * j))
                logits_sb = hw.tile([128, 128], f32)
                nc.vector.tensor_tensor(out=logits_sb[:, :], in0=log_ps[:, :],
                                        in1=b2p[:, :], op=OP.add)
                nc.sync.dma_start(out=o_logits[:, :], in_=logits_sb[:, :])

                ptr = hps.tile([128, 128], f32, tag="tr")
                nc.tensor.transpose(ptr[:, :], logits_sb[:, :], identF[:, :])
                tr_sb = hw.tile([128, 128], f32)
                nc.vector.tensor_copy(tr_sb[:, :], ptr[:, :])
                e_sb = hw.tile([128, NB * ODIM], f32)
                nc.scalar.activation(e_sb[:, :],
                                     _ap(tr_sb, 0, [[32, 4], [1, 3]]), AF.Exp)
                s4 = hw.tile([128, NB], f32)
                nc.vector.tensor_tensor(out=s4[:, :],
                                        in0=_ap(e_sb, 0, [[3, 4]]),
                                        in1=_ap(e_sb, 1, [[3, 4]]), op=OP.add)
                nc.vector.tensor_tensor(out=s4[:, :], in0=s4[:, :],
                                        in1=_ap(e_sb, 2, [[3, 4]]), op=OP.add)
                r4 = hw.tile([128, NB], f32)
                nc.vector.reciprocal(r4[:, :], s4[:, :])
                probs_sb = hw.tile([128, NB * ODIM], f32)
                nc.vector.tensor_tensor(out=probs_sb[:, :], in0=e_sb[:, :],
                                        in1=_ap(r4, 0, [[1, 4], [0, 3]]),
                                        op=OP.mult)
                nc.sync.dma_start(out=o_probs[:, :], in_=probs_sb[:, :])

    return nc


def host_inputs(x, edge_index, W_emb, b_emb, conv_w, conv_b, bn_gamma, bn_beta,
                bn_mean, bn_var, gat_W, gat_a_src, gat_a_dst, ew_W1, ew_b1,
                ew_W2, ew_b2, head_W1, head_b1, head_W2, head_b2):
    f = np.float32
    bf = bfloat16
    xs = np.asarray(x, f)[:, :, T - W:, :]                      # (B,N,15,64)
    xt = np.ascontiguousarray(np.transpose(xs, (3, 0, 1, 2)))   # (64,B,N,15)

    ei = np.asarray(edge_index)
    C = np.zeros((N, N), f)
    np.add.at(C, (ei[1].astype(np.int64), ei[0].astype(np.int64)), 1.0)
    CT = C.T.copy()                                             # [src, dst]

    W_emb = np.asarray(W_emb, f); b_emb = np.asarray(b_emb, f)
    cw = np.asarray(conv_w, f); cb = np.asarray(conv_b, f)
    inv = np.asarray(bn_gamma, f) / np.sqrt(np.asarray(bn_var, f) + BN_EPS)
    sc_all = inv.T.copy()                                       # (H,3)
    beff = cb.copy()
    beff[0] = beff[0] + (cw[0].sum(axis=2) @ b_emb)
    bi_all = ((beff - np.asarray(bn_mean, f)) * inv
              + np.asarray(bn_beta, f)).T.copy()                # (H,3)
    cw1 = np.concatenate(
        [(cw[0, :, :, k] @ W_emb).T for k in range(3)], axis=1)  # (64, 384)
    cw23 = np.concatenate(
        [cw[i, :, :, k].T for i in (1, 2) for k in range(3)], axis=1)

    ew_W1 = np.asarray(ew_W1, f)
    w2pad = np.zeros((H, 32), f)
    w2pad[:, 0] = np.asarray(ew_W2, f).reshape(H)

    gat_W = np.asarray(gat_W, f)
    ascol = np.stack([np.asarray(gat_a_src, f)[i, 0] for i in range(3)],
                     axis=1)                                    # (H,3)
    adcol = np.stack([np.asarray(gat_a_dst, f)[i, 0] for i in range(3)],
                     axis=1)

    hW1 = np.asarray(head_W1, f); hW2 = np.asarray(head_W2, f)
    hb1 = np.asarray(head_b1, f); hb2 = np.asarray(head_b2, f)
    b1p = np.zeros((128, 512), f)
    for p in range(64):
        b1p[0:64, p * 8:p * 8 + 4] = hb1[2 * p][:, None]
        b1p[64:128, p * 8 + 4:p * 8 + 8] = hb1[2 * p + 1][:, None]
    hW2T = np.concatenate([hW2[a].T for a in range(A)], axis=1)  # (64, 384)
    hW2Td = np.concatenate([hW2T, hW2T], axis=0)                 # (128, 384)
    b2p = np.zeros((128, 128), f)
    for j in range(4):
        for o in range(3):
            for q in range(32):
                b2p[32 * j + o, 4 * q:4 * q + 4] = hb2[4 * q + j, o]

    shared = {
        "cw1": cw1.astype(bf),
        "cw23": np.ascontiguousarray(cw23).astype(bf),
        "sc_all": np.ascontiguousarray(sc_all),
        "bi_all": np.ascontiguousarray(bi_all),
        "W1aT": np.ascontiguousarray(ew_W1[:, :H].T).astype(bf),
        "W1bT": np.ascontiguousarray(ew_W1[:, H:].T).astype(bf),
        "b1": np.asarray(ew_b1, f).reshape(H, 1),
        "w2pad": w2pad.astype(bf),
        "b2ew": np.asarray(ew_b2, f).reshape(1, 1),
        "CTA": np.ascontiguousarray(CT[:128]),
        "CTB": np.ascontiguousarray(CT[128:]),
        "gWT": np.ascontiguousarray(
            np.concatenate([gat_W[i].T for i in range(3)], axis=1)).astype(bf),
        "ascol": np.ascontiguousarray(ascol).astype(bf),
        "adcol": np.ascontiguousarray(adcol).astype(bf),
        "hW1T": np.ascontiguousarray(
            np.concatenate([hW1[a].T for a in range(A)], axis=1)).astype(bf),
        "b1p": b1p,
        "hW2Td": np.ascontiguousarray(hW2Td).astype(bf),
        "b2p": b2p,
    }
    in_maps = []
    for c in range(NC_CORES):
        m = dict(shared)
        m["xt"] = np.ascontiguousarray(
            xt[:, c * NB:(c + 1) * NB].reshape(DIN, NCOL * W)).astype(bf)
        in_maps.append(m)
    return in_maps


_CACHE = {}


def kernel(**inputs):
    _apply_sync_split_patch()
    if "nc" not in _CACHE:
        _CACHE["nc"] = build_program()
    nc = _CACHE["nc"]
    in_maps = host_inputs(**inputs)
    res = run_bass_kernel_spmd(nc, in_maps, list(range(NC_CORES)), trace=False)
    logits = np.empty((B, A, ODIM), np.float32)
    probs = np.empty((B, A, ODIM), np.float32)
    for c in range(NC_CORES):
        lg = res.results[c]["logits"]          # (128, 128)
        pr = res.results[c]["probs"]           # (128, 12)
        # logits[bi, 4q+j, o] = lg[32j+o, 4q+bi]
        lgr = lg.reshape(4, 32, 32, 4)[:, :3]  # [j, o, q, c]
        logits[c * NB:(c + 1) * NB] = (
            lgr.transpose(3, 2, 0, 1).reshape(NB, A, ODIM))
        # probs[bi, 4q+j, o] = pr[4q+c, 3j+o]
        prr = pr.reshape(32, 4, 4, 3)          # [q, c, j, o]
        probs[c * NB:(c + 1) * NB] = (
            prr.transpose(1, 0, 2, 3).reshape(NB, A, ODIM))
    return logits, probs
